# revision 2
# baseline (speedup 1.0000x reference)
"""Multi-head attention (B=4, S=2048, D=1280, H=10, hd=128) on 8 TRN2 NeuronCores.

Sharding: core c handles batch b = c//2, heads h0 = 5*(c%2)..+5 (data-parallel
batch x head-parallel TP). Row-sharded out-projection; partials summed on host.

fp8 strategy (e4m3 + DoubleRow matmuls = 2x PE throughput):
  - QK projection: hi/lo-split fp8 ("hilo3": xh Wh + xl Wh2 + xh Wl) with
    power-of-2 pre-scaling -> near-bf16 accuracy at 0.75x bf16 PE cost.
  - Q,K stored fp8 in a [64-partition, 2-group] layout so the S=K Q^T matmul
    (contraction hd=128 = 64x2) runs as fp8 DoubleRow at 2x.
  - V projection: 2-chain fp8 (xh(Wvh+Wvl)); V stored fp8 token-major. The
    token-mean of all V-path quantization error is removed by a rank-1
    correction MLO = xbar@Wv (host, f32) - sum_j(v8_j)/S (device, exact).
  - P = exp(scale*s): ACT writes bf16, DVE computes p' = p-1 into fp8
    (centering keeps quantization noise ~8x below uncentered fp8).
    PV and the colsum Z run as fp8 DoubleRow on PE:
      num = sum_j v8_j  (+)  P'^T V8    (PSUM accumulate, VS8-broadcast init)
      Z   = 2048 (+) sum_j p'_j         (ones-DR chain + K=1 const matmul)
      out_head = num/Z + MLO
  - out-projection stays bf16 (direct-error path, fp8 would break tolerance).
All PSUM accumulation f32. Expected rel err ~1.4e-2 (tolerance 2e-2).
"""

from collections import deque

import numpy as np

B, S, D = 4, 2048, 1280
HEADS = 10
HD = 128
NH = 5               # heads per core
P = 128
SCALE = float(D) ** -0.5
KT_D = D // P        # 10 k-tiles over D
NJT = S // P         # 16 j-tiles
NIC = S // 512       # 4 i-chunks
NTT = S // P         # 16 token-tiles for V projection

SXL = 64.0           # x-lo pre-scale (power of 2)
LAG = 3              # slots between exp(slot) and PV(slot)

_PROGRAM_CACHE = {}


def _build_program(swqk, swv, repeat=1):
    key = (swqk, swv, repeat)
    if key in _PROGRAM_CACHE:
        return _PROGRAM_CACHE[key]

    import concourse.mybir as mybir
    from concourse import bacc
    import concourse.tile as tile

    F32 = mybir.dt.float32
    F32R = mybir.dt.float32r
    BF16 = mybir.dt.bfloat16
    FP8 = mybir.dt.float8e4
    EXP = mybir.ActivationFunctionType.Exp
    DR = mybir.MatmulPerfMode.DoubleRow

    nc = bacc.Bacc()
    xhl_d = nc.declare_dram_parameter("xhl", [2, D, S], FP8, isOutput=False)
    wqkA_d = nc.declare_dram_parameter("wqkA", [P, KT_D, 2 * NH * HD], FP8, isOutput=False)
    wqkX_d = nc.declare_dram_parameter("wqkX", [P, KT_D, 2, 2 * NH * HD], FP8, isOutput=False)
    wvA_d = nc.declare_dram_parameter("wvA", [P, KT_D, NH * HD], FP8, isOutput=False)
    wvL_d = nc.declare_dram_parameter("wvL", [P, KT_D, NH * HD], FP8, isOutput=False)
    wo_d = nc.declare_dram_parameter("wout", [P, NH, D], BF16, isOutput=False)
    mv_d = nc.declare_dram_parameter("mvcol", [P, NH], F32, isOutput=False)
    out_d = nc.declare_dram_parameter("outT", [D, S], BF16, isOutput=True)

    xhl_t = xhl_d[:].rearrange("g (kt p) s -> p kt g s", p=P)     # [128,10,2,2048]
    xh_t = xhl_d[0].rearrange("(kt p) s -> p kt s", p=P)          # [128,10,2048]
    wqkA_t = wqkA_d[:]
    wqkX_t = wqkX_d[:]
    wvA_t = wvA_d[:]
    wvL_t = wvL_d[:]
    wo_t = wo_d[:]

    with tile.TileContext(nc) as tc:
        with (
            tc.tile_pool(name="persist", bufs=1) as persist,
            tc.tile_pool(name="xio", bufs=2) as xio,
            tc.tile_pool(name="vio", bufs=2) as vio,
            tc.tile_pool(name="tmp8p", bufs=2) as tmp8p,
            tc.tile_pool(name="pt2p", bufs=2) as pt2p,
            tc.tile_pool(name="pt8p", bufs=8 * (LAG + 1)) as pt8p,
            tc.tile_pool(name="otp", bufs=2) as otp,
            tc.tile_pool(name="rowp", bufs=3) as rowp,
            tc.tile_pool(name="outcp", bufs=2) as outcp,
            tc.tile_pool(name="ps_s", bufs=2, space="PSUM") as ps_s,
            tc.tile_pool(name="ps_oz", bufs=2, space="PSUM") as ps_oz,
            tc.tile_pool(name="ps_w", bufs=2, space="PSUM") as ps_w,
        ):
            QT8 = persist.tile([64, 2, NH, S], FP8, name="QT8")
            KT8 = persist.tile([64, 2, NH, S], FP8, name="KT8")
            V8 = persist.tile([P, NJT, NH, HD], FP8, name="V8")
            WQA = persist.tile([P, KT_D, 2 * NH * HD], FP8, name="WQA")
            WQX = persist.tile([P, KT_D, 2, 2 * NH * HD], FP8, name="WQX")
            WVA = persist.tile([P, KT_D, NH * HD], FP8, name="WVA")
            WVL = persist.tile([P, KT_D, NH * HD], FP8, name="WVL")
            WO = persist.tile([P, NH, D], BF16, name="WO")
            ones8 = persist.tile([P, 2, HD], FP8, name="ones8")
            mvcol = persist.tile([P, NH, 1], F32, name="mvcol")
            mloT = persist.tile([P, NH, 1], F32, name="mloT")
            VS8col = persist.tile([P, NH, 1], F32, name="VS8col")
            ident = persist.tile([P, P], F32, name="ident")


            from concourse.masks import make_identity
            nc.gpsimd.memset(ones8[:], 1.0)
            make_identity(nc, ident[:])
            nc.sync.dma_start(mvcol[:].rearrange("p h o -> p (h o)"), mv_d[:])

            _loaded = set()
            for rep in range(repeat):
                # lazy weight-chunk DMAs: emitted at first use so the serial
                # DMA engine services them in compute-critical order
                def need_w(kind, m=None):
                    tag = (kind, m)
                    if tag in _loaded:
                        return
                    _loaded.add(tag)
                    if kind == "qk":
                        ms = slice(m * P, (m + 1) * P)
                        nc.sync.dma_start(WQA[:, :, ms], wqkA_t[:, :, ms])
                        nc.sync.dma_start(WQX[:, :, :, ms],
                                          wqkX_t[:, :, :, ms])
                    elif kind == "wv":
                        nc.sync.dma_start(WVA[:, m:m + 2, :],
                                          wvA_t[:, m:m + 2, :])
                        nc.sync.dma_start(WVL[:, m:m + 2, :],
                                          wvL_t[:, m:m + 2, :])
                    elif kind == "wo":
                        nc.sync.dma_start(WO[:, m, :], wo_t[:, m, :])

                # ================= helpers =================
                def stage_xhl(ic):
                    isl = slice(ic * 512, (ic + 1) * 512)
                    xsb = xio.tile([P, KT_D, 2, 512], FP8, name="xhl")
                    nc.scalar.dma_start(xsb[:, :, 0, :], xhl_t[:, :, 0, isl])
                    nc.scalar.dma_start(xsb[:, :, 1, :], xhl_t[:, :, 1, isl])
                    return xsb

                def qk_proj(which, h, ic, xhl_sb):
                    """one m-tile of Q(which=0)/K(which=1) proj for (h, ic)."""
                    isl = slice(ic * 512, (ic + 1) * 512)
                    m = which * NH + h
                    ms = slice(m * P, (m + 1) * P)
                    need_w("qk", m)
                    q_ps = ps_w.tile([P, 512], F32, name="pw")
                    for tp in range(5):
                        nc.tensor.matmul(
                            q_ps[:], WQA[:, 2 * tp:2 * tp + 2, ms],
                            xhl_sb[:, 2 * tp:2 * tp + 2, 0, :],
                            start=(tp == 0), stop=False, perf_mode=DR)
                    for kt in range(KT_D):
                        nc.tensor.matmul(
                            q_ps[:], WQX[:, kt, :, ms], xhl_sb[:, kt, :, :],
                            start=False, stop=(kt == KT_D - 1), perf_mode=DR)
                    tmp8 = tmp8p.tile([P, 512], FP8, name="tmp8")
                    nc.vector.tensor_scalar_mul(tmp8[:], q_ps[:], 1.0 / swqk)
                    dst = KT8 if which else QT8
                    nc.sync.dma_start(dst[:, 0, h, isl], tmp8[0:64, :])
                    nc.sync.dma_start(dst[:, 1, h, isl], tmp8[64:128, :])

                def v_proj(tt):
                    """token-tile tt of the V projection (2-chain fp8 DR)."""
                    for tp in range(5):
                        need_w("wv", 2 * tp)
                    vt = vio.tile([P, KT_D, P], FP8, name="vt")
                    nc.scalar.dma_start(vt[:], xh_t[:, :, tt * P:(tt + 1) * P])
                    for g0, gn in ((0, 512), (512, 128)):
                        v_ps = ps_w.tile([P, 512], F32, name="pw")[:, :gn]
                        for tp in range(5):
                            nc.tensor.matmul(
                                v_ps, vt[:, 2 * tp:2 * tp + 2, :],
                                WVA[:, 2 * tp:2 * tp + 2, g0:g0 + gn],
                                start=(tp == 0), stop=False, perf_mode=DR)
                        for tp in range(5):
                            nc.tensor.matmul(
                                v_ps, vt[:, 2 * tp:2 * tp + 2, :],
                                WVL[:, 2 * tp:2 * tp + 2, g0:g0 + gn],
                                start=False, stop=(tp == 4), perf_mode=DR)
                        dst = V8[:, tt, g0 // HD:(g0 + gn) // HD, :]
                        nc.scalar.mul(dst, v_ps, 1.0 / swv)

                def vs8_unit():
                    """VS8col = sum_j v8 per head; mloT = mvcol - VS8col/2048."""
                    for h in range(NH):
                        vs_ps = ps_w.tile([P, 512], F32, name="pw")[:, :HD]
                        for t in range(NJT // 2):
                            nc.tensor.matmul(
                                vs_ps, ones8[:], V8[:, 2 * t:2 * t + 2, h, :],
                                start=(t == 0), stop=(t == NJT // 2 - 1),
                                perf_mode=DR)
                        vs_sb = rowp.tile([P, HD], F32, name="vs_sb")
                        nc.vector.tensor_copy(vs_sb[:], vs_ps)
                        t_ps = ps_w.tile([P, 512], F32, name="pw")[:, :HD]
                        nc.tensor.transpose(t_ps, vs_sb[:], ident[:])
                        nc.vector.tensor_copy(VS8col[:, h, :], t_ps[:, 0:1])
                    tcol = rowp.tile([P, NH, 1], F32, name="tcol")
                    nc.vector.tensor_scalar_mul(tcol[:], VS8col[:], -1.0 / 2048.0)
                    nc.vector.tensor_add(mloT[:], mvcol[:], tcol[:])

                def pv_unit(st):
                    """PV + Z chains for a pending slot (after exp+cast done)."""
                    ic, h, pt8s = st
                    o_ps = ps_oz.tile([P, 512], F32, name="oz")
                    z_ps = ps_oz.tile([P, 512], F32, name="oz")
                    for t in range(NJT // 2):
                        pair = pt8s[t][:].rearrange("p (g f) -> p g f", g=2)
                        nc.tensor.matmul(
                            o_ps[:], V8[:, 2 * t:2 * t + 2, h, :], pair,
                            start=(t == 0), stop=(t == NJT // 2 - 1),
                            perf_mode=DR)
                        nc.tensor.matmul(
                            z_ps[:], ones8[:], pair,
                            start=(t == 0), stop=(t == NJT // 2 - 1),
                            perf_mode=DR)
                    return (ic, h, o_ps, z_ps)

                def norm_unit(st):
                    """out_head = (o+VS8)/z + mlo -> OT[:, h, :] (bf16).

                    z_ps rows are all identical (M=128 ones-DR), so the
                    full-tile reciprocal IS the broadcast of 1/Z."""
                    ic, h, o_ps, z_ps = st
                    OT = OTs[ic]
                    zf = rowp.tile([P, 512], F32, name="zf")
                    nc.vector.tensor_scalar_add(zf[:], z_ps[:], 2048.0)
                    rec = rowp.tile([P, 512], F32, name="rec")
                    nc.vector.reciprocal(rec[:], zf[:])
                    t1 = rowp.tile([P, 512], F32, name="t1")
                    nc.vector.tensor_scalar_add(t1[:], o_ps[:], VS8col[:, h, :])
                    tmpb = rowp.tile([P, 512], BF16, name="tmpb")
                    nc.vector.tensor_mul(tmpb[:], t1[:], rec[:])
                    nc.vector.tensor_scalar_add(OT[:, h, :], tmpb[:], mloT[:, h, :])

                def out_proj(ic, m):
                    isl = slice(ic * 512, (ic + 1) * 512)
                    OT = OTs[ic]
                    for kt in range(NH):
                        need_w("wo", kt)
                    p_ps = ps_w.tile([P, 512], F32, name="pw")
                    for kt in range(NH):
                        nc.tensor.matmul(
                            p_ps[:], WO[:, kt, m * P:(m + 1) * P], OT[:, kt, :],
                            start=(kt == 0), stop=(kt == NH - 1))
                    outc = outcp.tile([P, 512], BF16, name="outc")
                    nc.scalar.copy(outc[:], p_ps[:])
                    nc.sync.dma_start(out_d[m * P:(m + 1) * P, isl], outc[:])

                # ================= emission =================
                # R-block: K projection for all (h, ic), then Q(ic0, h0).
                for ic in range(NIC):
                    xsb = stage_xhl(ic)
                    for h in range(NH):
                        qk_proj(1, h, ic, xsb)
                    if ic == 0:
                        qk_proj(0, 0, 0, xsb)

                # tagged filler queue: ("v", fn) must all run before first PV
                fillers = deque()

                def q0_rest():
                    xsb0 = stage_xhl(0)
                    for h in range(1, NH):
                        qk_proj(0, h, 0, xsb0)
                fillers.append(("q", q0_rest))
                for tt in range(NTT):
                    fillers.append(("v", lambda tt=tt: v_proj(tt)))
                fillers.append(("v", vs8_unit))

                def wo_prefetch():
                    for kt in range(NH):
                        need_w("wo", kt)
                fillers.append(("q", wo_prefetch))
                op_fill = {ic: deque() for ic in range(NIC)}

                def pop_filler():
                    if fillers:
                        fillers.popleft()[1]()
                        return
                    for ic in range(NIC):
                        if op_fill[ic]:
                            op_fill[ic].popleft()()
                            return

                def queue_op(ic):
                    for m in range(KT_D):
                        op_fill[ic].append(lambda ic=ic, m=m: out_proj(ic, m))

                pending_pv = deque()
                pending_norm = deque()
                OTs = {}

                slots = [(ic, h) for ic in range(NIC) for h in range(NH)]
                for j, (ic, h) in enumerate(slots):
                    isl = slice(ic * 512, (ic + 1) * 512)
                    if h == 0:
                        # OT buffer reuse (bufs=2): flush ic-2's out-proj first
                        if ic >= 2:
                            while op_fill[ic - 2]:
                                op_fill[ic - 2].popleft()()
                        OTs[ic] = otp.tile([P, NH, 512], BF16, name="OT")
                    if pending_norm:
                        st = pending_norm.popleft()
                        norm_unit(st)
                        if st[1] == NH - 1:
                            queue_op(st[0])
                    if h == 1 and ic + 1 < NIC:
                        # stage x now; spread the 5 Q-projections as fillers
                        xsb_n = stage_xhl(ic + 1)
                        for hh in range(NH):
                            fillers.append(
                                ("q", lambda hh=hh, icn=ic + 1, xs=xsb_n:
                                    qk_proj(0, hh, icn, xs)))
                    pt8s = []
                    for jp in range(8):
                        s_ps = ps_s.tile([P, 1024], F32, name="sp")
                        for half in range(2):
                            jt = 2 * jp + half
                            nc.tensor.matmul(
                                s_ps[:, half * 512:(half + 1) * 512],
                                KT8[:, :, h, jt * P:(jt + 1) * P],
                                QT8[:, :, h, isl],
                                start=True, stop=True, perf_mode=DR)
                        pt2 = pt2p.tile([P, 1024], BF16, name="pt2")
                        nc.scalar.activation(pt2[:], s_ps[:], EXP, scale=SCALE)
                        pt8 = pt8p.tile([P, 1024], FP8, name="pt8")
                        nc.vector.tensor_scalar_add(pt8[:], pt2[:], -1.0)
                        pt8s.append(pt8)
                        if jp % 2 == 1:
                            pop_filler()
                    pending_pv.append((ic, h, pt8s))
                    if j >= 14 and len(pending_pv) >= 2:
                        pending_norm.append(pv_unit(pending_pv.popleft()))
                        st = pending_norm.popleft()
                        norm_unit(st)
                        if st[1] == NH - 1:
                            queue_op(st[0])
                    if j >= LAG:
                        if j == LAG:
                            # PV needs the complete V8 + VS8row: force-drain
                            # all "v"-tagged fillers (keep later "q" fillers)
                            keep = deque()
                            while fillers:
                                tag, fn = fillers.popleft()
                                if tag == "v":
                                    fn()
                                else:
                                    keep.append((tag, fn))
                            fillers.extend(keep)
                        pending_norm.append(pv_unit(pending_pv.popleft()))
                    pop_filler()

                # drain
                while pending_pv:
                    pending_norm.append(pv_unit(pending_pv.popleft()))
                    st = pending_norm.popleft()
                    norm_unit(st)
                    if st[1] == NH - 1:
                        queue_op(st[0])
                    pop_filler()
                    pop_filler()
                while pending_norm:
                    st = pending_norm.popleft()
                    norm_unit(st)
                    if st[1] == NH - 1:
                        queue_op(st[0])
                for ic in range(NIC):
                    while op_fill[ic]:
                        op_fill[ic].popleft()()

    nc.finalize()
    _PROGRAM_CACHE[key] = nc
    return nc


def _p2scale(a, target=128.0):
    m = float(np.abs(a).max())
    if m <= 0:
        return 1.0
    return float(2.0 ** np.floor(np.log2(target / m)))


def _shard_inputs(x, w_qkv, w_out):
    """Host prep: per-core fp8 hi/lo tensors + scales (exact powers of 2)."""
    import ml_dtypes
    f8 = ml_dtypes.float8_e4m3
    bf16 = ml_dtypes.bfloat16

    def q8(a):
        return np.asarray(a, f8)

    def pmaj(a):
        """[D, ...] row-major -> [128, KT_D, ...] partition-major."""
        return np.ascontiguousarray(
            a.reshape(KT_D, P, *a.shape[1:]).swapaxes(0, 1))

    swqk = _p2scale(w_qkv[:, :2 * D])
    swv = _p2scale(w_qkv[:, 2 * D:])
    in_maps = []
    for c in range(8):
        b = c // 2
        h0 = NH * (c % 2)
        cq = w_qkv[:, 0 * D + h0 * HD:0 * D + (h0 + NH) * HD]
        ck = w_qkv[:, 1 * D + h0 * HD:1 * D + (h0 + NH) * HD]
        cv = w_qkv[:, 2 * D + h0 * HD:2 * D + (h0 + NH) * HD]
        wqk = np.concatenate([cq, ck], axis=1)            # [D, 1280]
        wqk_h8 = q8(wqk * swqk)
        wqk_h = wqk_h8.astype(np.float32)
        wqk_l8 = q8(wqk * swqk - wqk_h)
        wqk_h2 = q8(wqk_h / SXL)
        wqkX = np.stack([wqk_l8, wqk_h2], axis=1)         # [D, 2, 1280]
        wv_h8 = q8(cv * swv)
        wv_l8 = q8(cv * swv - wv_h8.astype(np.float32))
        xb = np.asarray(x[b], np.float32)                 # [S, D]
        xh8 = q8(xb)
        xl8 = q8((xb - xh8.astype(np.float32)) * SXL)
        xhl = np.stack([np.ascontiguousarray(xh8.T),
                        np.ascontiguousarray(xl8.T)], axis=0)  # [2, D, S]
        xbar = xb.mean(axis=0, dtype=np.float64).astype(np.float32)
        mv = (xbar.astype(np.float64) @ cv.astype(np.float64)
              ).astype(np.float32)                        # [640]
        mvcol = np.ascontiguousarray(mv.reshape(NH, HD).T)  # [128, NH]
        wo_c = w_out[h0 * HD:(h0 + NH) * HD, :].astype(bf16)   # [640, D]
        wo_pm = np.ascontiguousarray(
            wo_c.reshape(NH, P, D).swapaxes(0, 1))             # [128, 5, D]
        in_maps.append(dict(
            xhl=xhl,
            wqkA=pmaj(wqk_h8),
            wqkX=pmaj(wqkX),
            wvA=pmaj(wv_h8),
            wvL=pmaj(wv_l8),
            wout=wo_pm,
            mvcol=mvcol,
        ))
    return in_maps, swqk, swv


def run_sharded(x, w_qkv, w_out, b_out, repeat=1, trace=False):
    from concourse.bass_utils import run_bass_kernel_spmd

    in_maps, swqk, swv = _shard_inputs(x, w_qkv, w_out)
    nc = _build_program(swqk, swv, repeat)
    res = run_bass_kernel_spmd(nc, in_maps, list(range(8)), trace=trace)
    out = np.empty((B, S, D), np.float32)
    for b in range(B):
        out[b] = (res.results[2 * b]["outT"].T.astype(np.float32)
                  + res.results[2 * b + 1]["outT"].T.astype(np.float32)
                  + b_out[None, :])
    return out, res


def kernel(x, w_qkv, w_out, b_out):
    x = np.asarray(x, np.float32)
    w_qkv = np.asarray(w_qkv, np.float32)
    w_out = np.asarray(w_out, np.float32)
    b_out = np.asarray(b_out, np.float32)
    out, _ = run_sharded(x, w_qkv, w_out, b_out)
    return out


# revision 4
# speedup vs baseline: 1.0479x; 1.0479x over previous
"""Multi-head attention (B=4, S=2048, D=1280, H=10, hd=128) on 8 TRN2 NeuronCores.

Sharding: core c handles batch b = c//2, heads h0 = 5*(c%2)..+5 (data-parallel
batch x head-parallel TP). Row-sharded out-projection; partials summed on host.

fp8 strategy (e4m3 + DoubleRow matmuls = 2x PE throughput):
  - QK projection: hi/lo-split fp8 ("hilo3": xh Wh + xl Wh2 + xh Wl) with
    power-of-2 pre-scaling -> near-bf16 accuracy at 0.75x bf16 PE cost.
  - Q,K stored fp8 in a [64-partition, 2-group] layout so the S=K Q^T matmul
    (contraction hd=128 = 64x2) runs as fp8 DoubleRow at 2x.
  - V projection: 2-chain fp8 (xh(Wvh+Wvl)); V stored fp8 token-major. The
    token-mean of all V-path quantization error is removed by a rank-1
    correction MLO = xbar@Wv (host, f32) - sum_j(v8_j)/S (device, exact).
  - P = exp(scale*s): ACT writes bf16, DVE computes p' = p-1 into fp8
    (centering keeps quantization noise ~8x below uncentered fp8).
    PV and the colsum Z run as fp8 DoubleRow on PE:
      num = sum_j v8_j  (+)  P'^T V8    (PSUM accumulate, VS8-broadcast init)
      Z   = 2048 (+) sum_j p'_j         (ones-DR chain + K=1 const matmul)
      out_head = num/Z + MLO
  - out-projection stays bf16 (direct-error path, fp8 would break tolerance).
All PSUM accumulation f32. Expected rel err ~1.4e-2 (tolerance 2e-2).
"""

from collections import deque

import numpy as np

B, S, D = 4, 2048, 1280
HEADS = 10
HD = 128
NH = 5               # heads per core
P = 128
SCALE = float(D) ** -0.5
KT_D = D // P        # 10 k-tiles over D
NJT = S // P         # 16 j-tiles
NIC = S // 512       # 4 i-chunks
NTT = S // P         # 16 token-tiles for V projection

SXL = 64.0           # x-lo pre-scale (power of 2)
LAG = 3              # slots between exp(slot) and PV(slot)

_PROGRAM_CACHE = {}


def _build_program(swqk, swv, repeat=1):
    key = (swqk, swv, repeat)
    if key in _PROGRAM_CACHE:
        return _PROGRAM_CACHE[key]

    import concourse.mybir as mybir
    from concourse import bacc
    import concourse.tile as tile

    F32 = mybir.dt.float32
    F32R = mybir.dt.float32r
    BF16 = mybir.dt.bfloat16
    FP8 = mybir.dt.float8e4
    EXP = mybir.ActivationFunctionType.Exp
    DR = mybir.MatmulPerfMode.DoubleRow

    nc = bacc.Bacc()
    xhl_d = nc.declare_dram_parameter("xhl", [2, D, S], FP8, isOutput=False)
    wqkA_d = nc.declare_dram_parameter("wqkA", [P, KT_D, 2 * NH * HD], FP8, isOutput=False)
    wqkX_d = nc.declare_dram_parameter("wqkX", [P, KT_D, 2, 2 * NH * HD], FP8, isOutput=False)
    wvA_d = nc.declare_dram_parameter("wvA", [P, KT_D, NH * HD], FP8, isOutput=False)
    wvL_d = nc.declare_dram_parameter("wvL", [P, KT_D, NH * HD], FP8, isOutput=False)
    wo_d = nc.declare_dram_parameter("wout", [P, NH, D], BF16, isOutput=False)
    mv_d = nc.declare_dram_parameter("mvcol", [P, NH], F32, isOutput=False)
    out_d = nc.declare_dram_parameter("outT", [D, S], BF16, isOutput=True)

    xhl_t = xhl_d[:].rearrange("g (kt p) s -> p kt g s", p=P)     # [128,10,2,2048]
    xh_t = xhl_d[0].rearrange("(kt p) s -> p kt s", p=P)          # [128,10,2048]
    wqkA_t = wqkA_d[:]
    wqkX_t = wqkX_d[:]
    wvA_t = wvA_d[:]
    wvL_t = wvL_d[:]
    wo_t = wo_d[:]

    with tile.TileContext(nc) as tc:
        with (
            tc.tile_pool(name="persist", bufs=1) as persist,
            tc.tile_pool(name="xio", bufs=2) as xio,
            tc.tile_pool(name="vio", bufs=2) as vio,
            tc.tile_pool(name="tmp8p", bufs=6) as tmp8p,
            tc.tile_pool(name="pt2p", bufs=3) as pt2p,
            tc.tile_pool(name="pt8p", bufs=8 * (LAG + 1)) as pt8p,
            tc.tile_pool(name="otp", bufs=2) as otp,
            tc.tile_pool(name="rowp", bufs=3) as rowp,
            tc.tile_pool(name="outcp", bufs=2) as outcp,
            tc.tile_pool(name="ps_s", bufs=2, space="PSUM") as ps_s,
            tc.tile_pool(name="ps_oz", bufs=2, space="PSUM") as ps_oz,
            tc.tile_pool(name="ps_w", bufs=2, space="PSUM") as ps_w,
        ):
            QT8 = persist.tile([64, 2, NH, S], FP8, name="QT8")
            KT8 = persist.tile([64, 2, NH, S], FP8, name="KT8")
            V8 = persist.tile([P, NJT, NH, HD], FP8, name="V8")
            WQA = persist.tile([P, KT_D, 2 * NH * HD], FP8, name="WQA")
            WQX = persist.tile([P, KT_D, 2, 2 * NH * HD], FP8, name="WQX")
            WVA = persist.tile([P, KT_D, NH * HD], FP8, name="WVA")
            WVL = persist.tile([P, KT_D, NH * HD], FP8, name="WVL")
            WO = persist.tile([P, NH, D], BF16, name="WO")
            ones8 = persist.tile([P, 2, HD], FP8, name="ones8")
            eights8 = persist.tile([P, 2, HD], FP8, name="eights8")
            ones1k = persist.tile([P, 1024], FP8, name="ones1k")
            mvcol = persist.tile([P, NH, 1], F32, name="mvcol")
            mloT = persist.tile([P, NH, 1], F32, name="mloT")
            VS8col = persist.tile([P, NH, 1], F32, name="VS8col")
            ident = persist.tile([P, P], F32, name="ident")


            from concourse.masks import make_identity
            nc.gpsimd.memset(ones8[:], 1.0)
            nc.gpsimd.memset(eights8[:], 8.0)
            nc.gpsimd.memset(ones1k[:], 1.0)
            make_identity(nc, ident[:])
            nc.sync.dma_start(mvcol[:].rearrange("p h o -> p (h o)"), mv_d[:])

            _loaded = set()
            for rep in range(repeat):
                # lazy weight-chunk DMAs: emitted at first use so the serial
                # DMA engine services them in compute-critical order
                def need_w(kind, m=None):
                    tag = (kind, m)
                    if tag in _loaded:
                        return
                    _loaded.add(tag)
                    if kind == "qk":
                        ms = slice(m * P, (m + 1) * P)
                        nc.sync.dma_start(WQA[:, :, ms], wqkA_t[:, :, ms])
                        nc.sync.dma_start(WQX[:, :, :, ms],
                                          wqkX_t[:, :, :, ms])
                    elif kind == "wv":
                        nc.sync.dma_start(WVA[:, m:m + 2, :],
                                          wvA_t[:, m:m + 2, :])
                        nc.sync.dma_start(WVL[:, m:m + 2, :],
                                          wvL_t[:, m:m + 2, :])
                    elif kind == "wo":
                        nc.sync.dma_start(WO[:, m, :], wo_t[:, m, :])

                # ================= helpers =================
                def stage_xhl(ic):
                    isl = slice(ic * 512, (ic + 1) * 512)
                    xsb = xio.tile([P, KT_D, 2, 512], FP8, name="xhl")
                    nc.scalar.dma_start(xsb[:, :, 0, :], xhl_t[:, :, 0, isl])
                    nc.scalar.dma_start(xsb[:, :, 1, :], xhl_t[:, :, 1, isl])
                    return xsb

                def qk_proj(which, h, ic, xhl_sb):
                    """one m-tile of Q(which=0)/K(which=1) proj for (h, ic)."""
                    isl = slice(ic * 512, (ic + 1) * 512)
                    m = which * NH + h
                    ms = slice(m * P, (m + 1) * P)
                    need_w("qk", m)
                    q_ps = ps_w.tile([P, 512], F32, name="pw")
                    for tp in range(5):
                        nc.tensor.matmul(
                            q_ps[:], WQA[:, 2 * tp:2 * tp + 2, ms],
                            xhl_sb[:, 2 * tp:2 * tp + 2, 0, :],
                            start=(tp == 0), stop=False, perf_mode=DR)
                    for kt in range(KT_D):
                        nc.tensor.matmul(
                            q_ps[:], WQX[:, kt, :, ms], xhl_sb[:, kt, :, :],
                            start=False, stop=(kt == KT_D - 1), perf_mode=DR)
                    tmp8 = tmp8p.tile([P, 512], FP8, name="tmp8")
                    nc.vector.tensor_scalar_mul(tmp8[:], q_ps[:], 1.0 / swqk)
                    dst = KT8 if which else QT8
                    nc.sync.dma_start(dst[:, 0, h, isl], tmp8[0:64, :])
                    nc.sync.dma_start(dst[:, 1, h, isl], tmp8[64:128, :])

                def v_proj(tt):
                    """token-tile tt of the V projection (2-chain fp8 DR)."""
                    for tp in range(5):
                        need_w("wv", 2 * tp)
                    vt = vio.tile([P, KT_D, P], FP8, name="vt")
                    nc.scalar.dma_start(vt[:], xh_t[:, :, tt * P:(tt + 1) * P])
                    for g0, gn in ((0, 512), (512, 128)):
                        v_ps = ps_w.tile([P, 512], F32, name="pw")[:, :gn]
                        for tp in range(5):
                            nc.tensor.matmul(
                                v_ps, vt[:, 2 * tp:2 * tp + 2, :],
                                WVA[:, 2 * tp:2 * tp + 2, g0:g0 + gn],
                                start=(tp == 0), stop=False, perf_mode=DR)
                        for tp in range(5):
                            nc.tensor.matmul(
                                v_ps, vt[:, 2 * tp:2 * tp + 2, :],
                                WVL[:, 2 * tp:2 * tp + 2, g0:g0 + gn],
                                start=False, stop=(tp == 4), perf_mode=DR)
                        dst = V8[:, tt, g0 // HD:(g0 + gn) // HD, :]
                        nc.scalar.mul(dst, v_ps, 1.0 / swv)

                def vs8_unit():
                    """VS8col = sum_j v8 per head; mloT = mvcol - VS8col/2048."""
                    for h in range(NH):
                        vs_ps = ps_w.tile([P, 512], F32, name="pw")[:, :HD]
                        for t in range(NJT // 2):
                            nc.tensor.matmul(
                                vs_ps, ones8[:], V8[:, 2 * t:2 * t + 2, h, :],
                                start=(t == 0), stop=(t == NJT // 2 - 1),
                                perf_mode=DR)
                        vs_sb = rowp.tile([P, HD], F32, name="vs_sb")
                        nc.vector.tensor_copy(vs_sb[:], vs_ps)
                        t_ps = ps_w.tile([P, 512], F32, name="pw")[:, :HD]
                        nc.tensor.transpose(t_ps, vs_sb[:], ident[:])
                        nc.vector.tensor_copy(VS8col[:, h, :], t_ps[:, 0:1])
                    tcol = rowp.tile([P, NH, 1], F32, name="tcol")
                    nc.vector.tensor_scalar_mul(tcol[:], VS8col[:], -1.0 / 2048.0)
                    nc.vector.tensor_add(mloT[:], mvcol[:], tcol[:])

                def pv_unit(st):
                    """PV + Z chains for a pending slot (after exp+cast done)."""
                    ic, h, pt8s = st
                    o_ps = ps_oz.tile([P, 512], F32, name="oz")
                    z_ps = ps_oz.tile([P, 512], F32, name="oz")
                    onepair = ones1k[:].rearrange("p (g f) -> p g f", g=2)
                    nc.tensor.matmul(z_ps[:], eights8[:], onepair,
                                     start=True, stop=False, perf_mode=DR)
                    for t in range(NJT // 2):
                        pair = pt8s[t][:].rearrange("p (g f) -> p g f", g=2)
                        nc.tensor.matmul(
                            o_ps[:], V8[:, 2 * t:2 * t + 2, h, :], pair,
                            start=(t == 0), stop=(t == NJT // 2 - 1),
                            perf_mode=DR)
                        nc.tensor.matmul(
                            z_ps[:], ones8[:], pair,
                            start=False, stop=(t == NJT // 2 - 1),
                            perf_mode=DR)
                    return (ic, h, o_ps, z_ps)

                def norm_unit(st):
                    """out_head = (o+VS8)/z + mlo -> OT[:, h, :] (bf16).

                    z_ps rows are all identical (M=128 ones-DR), so the
                    full-tile reciprocal IS the broadcast of 1/Z."""
                    ic, h, o_ps, z_ps = st
                    OT = OTs[ic]
                    rec = rowp.tile([P, 512], F32, name="rec")
                    nc.vector.reciprocal(rec[:], z_ps[:])
                    t1 = rowp.tile([P, 512], F32, name="t1")
                    nc.vector.tensor_scalar_add(t1[:], o_ps[:], VS8col[:, h, :])
                    tmpb = rowp.tile([P, 512], BF16, name="tmpb")
                    nc.vector.tensor_mul(tmpb[:], t1[:], rec[:])
                    nc.vector.tensor_scalar_add(OT[:, h, :], tmpb[:], mloT[:, h, :])

                def out_proj(ic, m):
                    isl = slice(ic * 512, (ic + 1) * 512)
                    OT = OTs[ic]
                    for kt in range(NH):
                        need_w("wo", kt)
                    p_ps = ps_w.tile([P, 512], F32, name="pw")
                    for kt in range(NH):
                        nc.tensor.matmul(
                            p_ps[:], WO[:, kt, m * P:(m + 1) * P], OT[:, kt, :],
                            start=(kt == 0), stop=(kt == NH - 1))
                    outc = outcp.tile([P, 512], BF16, name="outc")
                    nc.scalar.copy(outc[:], p_ps[:])
                    nc.sync.dma_start(out_d[m * P:(m + 1) * P, isl], outc[:])

                # ================= emission =================
                # R-block: K projection for all (h, ic), then Q(ic0, h0).
                for ic in range(NIC):
                    xsb = stage_xhl(ic)
                    for h in range(NH):
                        qk_proj(1, h, ic, xsb)
                    if ic == 0:
                        qk_proj(0, 0, 0, xsb)

                # tagged filler queue: ("v", fn) must all run before first PV
                fillers = deque()

                def q0_rest():
                    xsb0 = stage_xhl(0)
                    for h in range(1, NH):
                        qk_proj(0, h, 0, xsb0)
                fillers.append(("q", q0_rest))
                for tt in range(NTT):
                    fillers.append(("v", lambda tt=tt: v_proj(tt)))
                fillers.append(("v", vs8_unit))

                def wo_prefetch():
                    for kt in range(NH):
                        need_w("wo", kt)
                fillers.append(("q", wo_prefetch))
                op_fill = {ic: deque() for ic in range(NIC)}

                def pop_filler():
                    if fillers:
                        fillers.popleft()[1]()
                        return
                    for ic in range(NIC):
                        if op_fill[ic]:
                            op_fill[ic].popleft()()
                            return

                def queue_op(ic):
                    for m in range(KT_D):
                        op_fill[ic].append(lambda ic=ic, m=m: out_proj(ic, m))

                pending_pv = deque()
                pending_norm = deque()
                OTs = {}

                slots = [(ic, h) for ic in range(NIC) for h in range(NH)]
                for j, (ic, h) in enumerate(slots):
                    isl = slice(ic * 512, (ic + 1) * 512)
                    if h == 0:
                        # OT buffer reuse (bufs=2): flush ic-2's out-proj first
                        if ic >= 2:
                            while op_fill[ic - 2]:
                                op_fill[ic - 2].popleft()()
                        OTs[ic] = otp.tile([P, NH, 512], BF16, name="OT")
                    if pending_norm:
                        st = pending_norm.popleft()
                        norm_unit(st)
                        if st[1] == NH - 1:
                            queue_op(st[0])
                    if h == 1 and ic + 1 < NIC:
                        # stage x now; spread the 5 Q-projections as fillers
                        xsb_n = stage_xhl(ic + 1)
                        for hh in range(NH):
                            fillers.append(
                                ("q", lambda hh=hh, icn=ic + 1, xs=xsb_n:
                                    qk_proj(0, hh, icn, xs)))
                    pt8s = []
                    for jp in range(8):
                        s_ps = ps_s.tile([P, 1024], F32, name="sp")
                        for half in range(2):
                            jt = 2 * jp + half
                            nc.tensor.matmul(
                                s_ps[:, half * 512:(half + 1) * 512],
                                KT8[:, :, h, jt * P:(jt + 1) * P],
                                QT8[:, :, h, isl],
                                start=True, stop=True, perf_mode=DR)
                        pt2 = pt2p.tile([P, 1024], BF16, name="pt2")
                        nc.scalar.activation(pt2[:], s_ps[:], EXP, scale=SCALE)
                        pt8 = pt8p.tile([P, 1024], FP8, name="pt8")
                        nc.vector.tensor_scalar_add(pt8[:], pt2[:], -1.0)
                        pt8s.append(pt8)
                        if jp % 2 == 1:
                            pop_filler()
                    pending_pv.append((ic, h, pt8s))
                    if j >= 14 and len(pending_pv) >= 2:
                        pending_norm.append(pv_unit(pending_pv.popleft()))
                        st = pending_norm.popleft()
                        norm_unit(st)
                        if st[1] == NH - 1:
                            queue_op(st[0])
                    if j >= LAG:
                        if j == LAG:
                            # PV needs the complete V8 + VS8row: force-drain
                            # all "v"-tagged fillers (keep later "q" fillers)
                            keep = deque()
                            while fillers:
                                tag, fn = fillers.popleft()
                                if tag == "v":
                                    fn()
                                else:
                                    keep.append((tag, fn))
                            fillers.extend(keep)
                        pending_norm.append(pv_unit(pending_pv.popleft()))
                    pop_filler()

                # drain
                while pending_pv:
                    pending_norm.append(pv_unit(pending_pv.popleft()))
                    st = pending_norm.popleft()
                    norm_unit(st)
                    if st[1] == NH - 1:
                        queue_op(st[0])
                    pop_filler()
                    pop_filler()
                while pending_norm:
                    st = pending_norm.popleft()
                    norm_unit(st)
                    if st[1] == NH - 1:
                        queue_op(st[0])
                for ic in range(NIC):
                    while op_fill[ic]:
                        op_fill[ic].popleft()()

    nc.finalize()
    _PROGRAM_CACHE[key] = nc
    return nc


def _p2scale(a, target=128.0):
    m = float(np.abs(a).max())
    if m <= 0:
        return 1.0
    return float(2.0 ** np.floor(np.log2(target / m)))


def _shard_inputs(x, w_qkv, w_out):
    """Host prep: per-core fp8 hi/lo tensors + scales (exact powers of 2)."""
    import ml_dtypes
    f8 = ml_dtypes.float8_e4m3
    bf16 = ml_dtypes.bfloat16

    def q8(a):
        return np.asarray(a, f8)

    def pmaj(a):
        """[D, ...] row-major -> [128, KT_D, ...] partition-major."""
        return np.ascontiguousarray(
            a.reshape(KT_D, P, *a.shape[1:]).swapaxes(0, 1))

    swqk = _p2scale(w_qkv[:, :2 * D])
    swv = _p2scale(w_qkv[:, 2 * D:])
    in_maps = []
    for c in range(8):
        b = c // 2
        h0 = NH * (c % 2)
        cq = w_qkv[:, 0 * D + h0 * HD:0 * D + (h0 + NH) * HD]
        ck = w_qkv[:, 1 * D + h0 * HD:1 * D + (h0 + NH) * HD]
        cv = w_qkv[:, 2 * D + h0 * HD:2 * D + (h0 + NH) * HD]
        wqk = np.concatenate([cq, ck], axis=1)            # [D, 1280]
        wqk_h8 = q8(wqk * swqk)
        wqk_h = wqk_h8.astype(np.float32)
        wqk_l8 = q8(wqk * swqk - wqk_h)
        wqk_h2 = q8(wqk_h / SXL)
        wqkX = np.stack([wqk_l8, wqk_h2], axis=1)         # [D, 2, 1280]
        wv_h8 = q8(cv * swv)
        wv_l8 = q8(cv * swv - wv_h8.astype(np.float32))
        xb = np.asarray(x[b], np.float32)                 # [S, D]
        xh8 = q8(xb)
        xl8 = q8((xb - xh8.astype(np.float32)) * SXL)
        xhl = np.stack([np.ascontiguousarray(xh8.T),
                        np.ascontiguousarray(xl8.T)], axis=0)  # [2, D, S]
        xbar = xb.mean(axis=0, dtype=np.float64).astype(np.float32)
        mv = (xbar.astype(np.float64) @ cv.astype(np.float64)
              ).astype(np.float32)                        # [640]
        mvcol = np.ascontiguousarray(mv.reshape(NH, HD).T)  # [128, NH]
        wo_c = w_out[h0 * HD:(h0 + NH) * HD, :].astype(bf16)   # [640, D]
        wo_pm = np.ascontiguousarray(
            wo_c.reshape(NH, P, D).swapaxes(0, 1))             # [128, 5, D]
        in_maps.append(dict(
            xhl=xhl,
            wqkA=pmaj(wqk_h8),
            wqkX=pmaj(wqkX),
            wvA=pmaj(wv_h8),
            wvL=pmaj(wv_l8),
            wout=wo_pm,
            mvcol=mvcol,
        ))
    return in_maps, swqk, swv


def run_sharded(x, w_qkv, w_out, b_out, repeat=1, trace=False):
    from concourse.bass_utils import run_bass_kernel_spmd

    in_maps, swqk, swv = _shard_inputs(x, w_qkv, w_out)
    nc = _build_program(swqk, swv, repeat)
    res = run_bass_kernel_spmd(nc, in_maps, list(range(8)), trace=trace)
    out = np.empty((B, S, D), np.float32)
    for b in range(B):
        out[b] = (res.results[2 * b]["outT"].T.astype(np.float32)
                  + res.results[2 * b + 1]["outT"].T.astype(np.float32)
                  + b_out[None, :])
    return out, res


def kernel(x, w_qkv, w_out, b_out):
    x = np.asarray(x, np.float32)
    w_qkv = np.asarray(w_qkv, np.float32)
    w_out = np.asarray(w_out, np.float32)
    b_out = np.asarray(b_out, np.float32)
    out, _ = run_sharded(x, w_qkv, w_out, b_out)
    return out
